# revision 2
# baseline (speedup 1.0000x reference)
"""AWD-LSTM forward on 8 Trainium2 NeuronCores — v2 "warm" design.

Sharding (unchanged from v1): 8 cores = 4 pairs; even core of a pair runs
the forward direction, odd the backward (fed time-reversed inputs so both
are forward scans).  Batch (32) is sharded 8 rows per pair.  Layer-1 input
needs both directions' layer-0 sequences, exchanged with a pairwise
AllGather.  The tiny fc+mish head runs on host.

v2 changes (vs the 16.3 ms v1):
  - The input projections (pre = x @ W_ih^T + bias) are FUSED into the
    recurrence loop: each step's tail (the ACT/DVE gate chain, ~1.5 us of
    PE idle in v1) is filled with a few projection matmuls that compute
    pre for steps ~16-32 ahead, into SBUF ping-pong chunk buffers.  This
    keeps the PE busy so the HAM clock gate stays at K=8/8 (2.4 GHz) —
    v1 ran throttled at 1.2 GHz for 94% of the kernel — and absorbs the
    ~0.7 ms standalone projection phases.
  - The 16 per-step pre-fold transpose-matmuls are gone: pre enters the
    gate PSUM banks via a DVE value-overwrite while the accumulation
    group stays open forever (one dummy start=True matmul at init; all
    gate matmuls start=False).  has_written bits stay set, so matmuls
    accumulate on top of the DVE-written pre values.
  - W_hh is fp8-e4m3 (stationary side only; h stays bf16): FWL loads
    fp8 weights 2x faster than bf16.  Everything is pre-scaled by 16 on
    host (clears e4m3's subnormal floor) and descaled for free via the
    ACT affine (scale=1/16).  Measured fp8 impact on final error vs
    fp32 reference: 6.7e-3 (budget 2e-2).
  - Gate banks laid out [i|g] and [f|o] so the exposed tail chain is
    sigmoid(f,o) -> f*c -> +i*tanh(g) -> tanh(c) -> o*tanh(c), with the
    i/g activations hidden under the f/o matmuls.
"""

import sys

sys.path.insert(0, "/opt/trn_rl_repo")

import numpy as np

import concourse.bass as bass
import concourse.bacc as bacc
import concourse.mybir as mybir
import concourse.tile as tile
from concourse.bass import ds

B, S, E = 32, 1024, 512
H = E
G4 = 4 * H            # 2048
BS = 8                # batch rows per pair
NCORES = 8
CH = 16               # steps per pre chunk
UNROLL = 64           # steps per loop body (4 chunks)
PAD = 2 * CH * BS     # x/seq column padding (2 chunks of lookahead slop)
F32 = mybir.dt.float32
BF16 = mybir.dt.bfloat16
FP8 = mybir.dt.float8e4
USE_FP8 = True

AF = mybir.ActivationFunctionType
MULT = mybir.AluOpType.mult
ADD = mybir.AluOpType.add

# mslot s (production/bias/pre-buffer order) -> real m-chunk (gate dim / 128)
# slots 0-7 -> bank_ig [i0..3, g0..3]; slots 8-15 -> bank_fo [f0..3, o0..3]
MORDER = [0, 1, 2, 3, 8, 9, 10, 11, 4, 5, 6, 7, 12, 13, 14, 15]


def build_program(T=S, fp8=USE_FP8):
    TB = T * BS
    TBP = TB + PAD
    WDT = FP8 if fp8 else BF16
    nc = bacc.Bacc(None, target_bir_lowering=False)

    # ---- I/O ----
    x_T = nc.dram_tensor("x_T", [4, 128, TBP], BF16, kind="ExternalInput")
    w0T = nc.dram_tensor("w0T", [4, 128, G4], BF16, kind="ExternalInput")
    b0T = nc.dram_tensor("b0T", [128, 16], F32, kind="ExternalInput")
    whh0T = nc.dram_tensor("whh0T", [4, 128, G4], WDT, kind="ExternalInput")
    w1oT = nc.dram_tensor("w1oT", [4, 128, G4], BF16, kind="ExternalInput")
    w1pT = nc.dram_tensor("w1pT", [4, 128, G4], BF16, kind="ExternalInput")
    b1T = nc.dram_tensor("b1T", [128, 16], F32, kind="ExternalInput")
    whh1T = nc.dram_tensor("whh1T", [4, 128, G4], WDT, kind="ExternalInput")
    pslot = nc.dram_tensor("pslot", [1, 1], mybir.dt.uint32, kind="ExternalInput")
    h1 = nc.dram_tensor("h1", [128, 32], BF16, kind="ExternalOutput")
    dbg_pre = nc.dram_tensor("dbg_pre", [128, 16, CH * BS], BF16, kind="ExternalOutput")
    dbg_loc = nc.dram_tensor("dbg_loc", [128, 4, 128], BF16, kind="ExternalOutput")

    with tile.TileContext(nc) as tc:
        with tc.tile_pool(name="dram", bufs=1, space="DRAM") as dram:
            locT = dram.tile([128, 4, TBP], BF16)
            revT = dram.tile([128, 4, TBP], BF16)
            ag = dram.tile([2, 128, 4, TBP], BF16)
            pbuf = dram.tile([128, 4, TBP], BF16)

            with (
                tc.tile_pool(name="outer", bufs=1) as op,
                tc.tile_pool(name="gpsum", bufs=1, space="PSUM") as gpsum,
                tc.tile_pool(name="ppsum", bufs=2, space="PSUM") as ppsum,
                tc.tile_pool(name="gtmp", bufs=2) as gp,
                tc.tile_pool(name="stage", bufs=4) as stp,
                tc.tile_pool(name="prebuf", bufs=1) as prep,
                tc.tile_pool(name="xin", bufs=2) as xp,
            ):
                bank_ig = gpsum.tile([128, 64], F32)
                bank_fo = gpsum.tile([128, 64], F32)
                cT = op.tile([128, 32], F32)
                hT = op.tile([128, 32], BF16)
                pres = [prep.tile([128, 16, CH * BS], BF16, name=f"pre{i}")
                        for i in range(4)]
                zlhs = op.tile([1, 128], BF16)
                zrhs = op.tile([1, 64], BF16)
                zslop = op.tile([128, 4, PAD], BF16)
                nc.gpsimd.memset(zlhs[:], 0.0)
                nc.gpsimd.memset(zrhs[:], 0.0)
                nc.gpsimd.memset(zslop[:], 0.0)
                nc.gpsimd.memset(cT[:], 0.0)
                nc.gpsimd.memset(hT[:], 0.0)

                # open the gate PSUM accumulation groups forever: write 0s with
                # start=True (sets has_written over the full [128, 64] region),
                # never issue stop.  All gate matmuls accumulate (start=False)
                # on top of DVE-prewritten pre values.
                nc.tensor.matmul(bank_ig[:], zlhs[:], zrhs[:],
                                 start=True, stop=False, skip_group_check=True)
                nc.tensor.matmul(bank_fo[:], zlhs[:], zrhs[:],
                                 start=True, stop=False, skip_group_check=True)

                # zero the lookahead slop so layer-1's projection reads are finite
                nc.sync.dma_start(locT[:, :, TB:TBP], zslop[:])
                nc.sync.dma_start(revT[:, :, TB:TBP], zslop[:])

                pools = dict(op=op, gp=gp, stp=stp, xp=xp, ppsum=ppsum,
                             bank_ig=bank_ig, bank_fo=bank_fo, cT=cT, hT=hT,
                             pres=pres)

                recur_layer(tc, pools, T, WDT,
                            xsrc=[(x_T, k) for k in range(4)],
                            wihT=[(w0T, k) for k in range(4)],
                            whhT=whh0T, biasT=b0T,
                            locT=locT, revT=revT, dbg_pre=dbg_pre)
                nc.sync.dma_start(dbg_loc[:], locT[:, :, 0:128])

                nc.gpsimd.collective_compute(
                    "AllGather",
                    mybir.AluOpType.bypass,
                    ins=[revT.opt()],
                    outs=[ag.opt()],
                    replica_groups=[[0, 1], [2, 3], [4, 5], [6, 7]],
                )

                # copy partner's gathered (pre-reversed) sequence to pbuf
                with tc.tile_pool(name="pslot", bufs=1) as pp:
                    pslot_sb = pp.tile([1, 1], mybir.dt.uint32)
                    nc.sync.dma_start(pslot_sb[:], pslot[:])
                    tmp_reg = nc.sync.alloc_register("pslot_reg")
                    nc.sync.reg_load(tmp_reg, pslot_sb[0:1, 0:1])
                    slot_reg = nc.sync.snap(tmp_reg, donate=True, min_val=0, max_val=1)
                    nc.sync.dma_start(
                        pbuf[:],
                        ag[ds(slot_reg, 1)].rearrange("s p k c -> (s p) k c"),
                    )

                nc.gpsimd.memset(cT[:], 0.0)
                nc.gpsimd.memset(hT[:], 0.0)

                recur_layer(tc, pools, T, WDT,
                            xsrc=[(locT, k) for k in range(4)]
                                 + [(pbuf, k) for k in range(4)],
                            wihT=[(w1oT, k) for k in range(4)]
                                 + [(w1pT, k) for k in range(4)],
                            whhT=whh1T, biasT=b1T)

                nc.sync.dma_start(h1[:], hT[:])

    nc.compile()
    return nc


def _xslice(src, k, col, n=128):
    """[128, n] moving slice at column `col` from an x-like source."""
    t, kk = src
    if len(t.shape) == 3 and t.shape[0] == 4:     # x_T [4, 128, TBP]
        return t[kk, :, col] if isinstance(col, slice) else t[kk, :, ds(col, n)]
    # locT/pbuf [128, 4, TBP]
    return t[:, kk, col] if isinstance(col, slice) else t[:, kk, ds(col, n)]


def recur_layer(tc, P, T, WDT, xsrc, wihT, whhT, biasT, locT=None, revT=None,
                dbg_pre=None):
    """One LSTM layer: fused projection + recurrence.

    xsrc: list of (dram_tensor, k) moving-operand sources, one per 128-wide
    contraction chunk (4 for layer 0, 4 own + 4 partner for layer 1).
    """
    nc = tc.nc
    nk = len(xsrc)
    store = locT is not None
    bank_ig, bank_fo = P["bank_ig"], P["bank_fo"]
    cT, hT, pres = P["cT"], P["hT"], P["pres"]
    gp, stp, xp, ppsum, op = P["gp"], P["stp"], P["xp"], P["ppsum"], P["op"]
    REV = 8 * T - 8

    with tc.tile_pool(name="wpool", bufs=1) as wp:
        whh_sb = []
        for k in range(4):
            w = wp.tile([128, G4], WDT, name=f"whh{k}")
            nc.sync.dma_start(w[:], whhT[k])
            whh_sb.append(w)
        wih_sb = []
        for j, (t, kk) in enumerate(wihT):
            w = wp.tile([128, G4], BF16, name=f"wih{j}")
            nc.sync.dma_start(w[:], t[kk])
            wih_sb.append(w)
        bias_sb = wp.tile([128, 16], F32)
        nc.sync.dma_start(bias_sb[:], biasT[:])

        def dma_xchunk(parity, col):
            """Fetch the [128,128] moving tiles for one chunk's projection."""
            xts = []
            for j in range(nk):
                xt = xp.tile([128, 128], BF16, name=f"x{j}", tag=f"x{j}p{parity}")
                nc.sync.dma_start(xt[:], _xslice(xsrc[j], j, col))
                xts.append(xt)
            return xts

        def emit_proj(s, xts, dst, pps):
            """Projection matmuls for mslot s into pps col 128*(s%4)."""
            m = MORDER[s]
            col = 128 * (s % 4)
            for j in range(nk):
                nc.tensor.matmul(pps[:, col:col + 128],
                                 wih_sb[j][:, 128 * m:128 * (m + 1)], xts[j][:],
                                 start=(j == 0), stop=(j == nk - 1))

        def emit_proj_copies(g, dst, pps):
            """Evacuate mslots 4g..4g+3 (+bias) from pps into dst."""
            for jj in range(4):
                s = 4 * g + jj
                nc.vector.tensor_scalar_add(dst[:, s, :],
                                            pps[:, 128 * jj:128 * jj + 128],
                                            bias_sb[:, s:s + 1])

        def prewrite(nxt_s):
            """DVE-overwrite both gate banks with pre for step nxt_s."""
            buf = pres[(nxt_s // CH) % 4]
            c0 = 8 * (nxt_s % CH)
            nc.vector.tensor_copy(
                bank_ig[:].rearrange("p (s b) -> p s b", s=8),
                buf[:, 0:8, c0:c0 + 8])
            nc.vector.tensor_copy(
                bank_fo[:].rearrange("p (s b) -> p s b", s=8),
                buf[:, 8:16, c0:c0 + 8])

        def emit_step(u, iv8, xts, dst, pps):
            """One LSTM step; consumes prewritten banks, prewrites step u+1.

            u: step index within the body (0..UNROLL-1); also emits the
            projection for mslot (u%CH) of the lookahead chunk into dst.
            """
            # Gate-block order [f, o, g, i]: the per-MM sem-inc stream
            # drains at ~41ns/inc (slower than the 32ns MM issue rate), so a
            # consumer of block B starts no earlier than (stream position of
            # B's last MM) x 41ns.  Putting f/o first lets sigmoid(f,o) and
            # btmp=f*c complete under the g/i stream; the exposed chain after
            # the last inc is only si -> a -> cT -> tanh(c) -> h.
            for j in range(4):
                m = 4 + j
                for k in range(4):
                    nc.tensor.matmul(bank_fo[:, 8 * j:8 * j + 8],
                                     whh_sb[k][:, 128 * m:128 * (m + 1)],
                                     hT[:, 8 * k:8 * k + 8],
                                     start=False, stop=False, skip_group_check=True)
            for j in range(4):
                m = 12 + j
                for k in range(4):
                    nc.tensor.matmul(bank_fo[:, 32 + 8 * j:40 + 8 * j],
                                     whh_sb[k][:, 128 * m:128 * (m + 1)],
                                     hT[:, 8 * k:8 * k + 8],
                                     start=False, stop=False, skip_group_check=True)
            sfo = gp.tile([128, 64], F32, tag="sfo")
            nc.scalar.activation(sfo[:], bank_fo[:], AF.Sigmoid, scale=0.0625)
            for j in range(4):
                m = 8 + j
                for k in range(4):
                    nc.tensor.matmul(bank_ig[:, 32 + 8 * j:40 + 8 * j],
                                     whh_sb[k][:, 128 * m:128 * (m + 1)],
                                     hT[:, 8 * k:8 * k + 8],
                                     start=False, stop=False, skip_group_check=True)
            tg = gp.tile([128, 32], F32, tag="tg")
            nc.scalar.activation(tg[:], bank_ig[:, 32:64], AF.Tanh, scale=0.0625)
            for j in range(4):
                for k in range(4):
                    nc.tensor.matmul(bank_ig[:, 8 * j:8 * j + 8],
                                     whh_sb[k][:, 128 * j:128 * (j + 1)],
                                     hT[:, 8 * k:8 * k + 8],
                                     start=False, stop=False, skip_group_check=True)
            si = gp.tile([128, 32], F32, tag="si")
            nc.scalar.activation(si[:], bank_ig[:, 0:32], AF.Sigmoid, scale=0.0625)

            buf = pres[((u + 1) // CH) % 4]
            c0 = 8 * ((u + 1) % CH)
            btmp = gp.tile([128, 32], F32, tag="btmp")
            nc.vector.tensor_tensor(btmp[:], sfo[:, 0:32], cT[:], MULT)
            nc.vector.tensor_copy(
                bank_fo[:].rearrange("p (s b) -> p s b", s=8),
                buf[:, 8:16, c0:c0 + 8])
            a = gp.tile([128, 32], F32, tag="a")
            nc.vector.tensor_tensor(a[:], si[:], tg[:], MULT)
            nc.vector.tensor_tensor(cT[:], a[:], btmp[:], ADD)
            nc.vector.tensor_copy(
                bank_ig[:].rearrange("p (s b) -> p s b", s=8),
                buf[:, 0:8, c0:c0 + 8])
            emit_proj(u % CH, xts, dst, pps)
            if u % 4 == 3:
                emit_proj_copies((u % CH) // 4, dst, pps)
            tct = gp.tile([128, 32], F32, tag="tct")
            nc.scalar.activation(tct[:], cT[:], AF.Tanh)
            nc.vector.tensor_tensor(hT[:], sfo[:, 32:64], tct[:], MULT)

            if store:
                st = stp.tile([128, 32], BF16, tag="st")
                nc.gpsimd.tensor_copy(st[:], hT[:])
                st3 = st[:].rearrange("p (k b) -> p k b", k=4)
                nc.sync.dma_start(locT[:, :, ds(iv8, 8)], st3)
                nc.gpsimd.dma_start(revT[:, :, ds(REV - iv8, 8)], st3)

        # ---- prologue: produce chunks 0-1 into pres[0:2], prewrite step 0 ----
        for c in range(2):
            xtsp = dma_xchunk(c, slice(128 * c, 128 * c + 128))
            for g in range(4):
                pps = ppsum.tile([128, 512], F32, tag="pj", name="pps")
                for jj in range(4):
                    emit_proj(4 * g + jj, xtsp, pres[c], pps)
                emit_proj_copies(g, pres[c], pps)
        prewrite(0)
        if dbg_pre is not None:
            nc.sync.dma_start(dbg_pre[:], pres[0][:])

        # ---- main loop: 32 steps (2 chunks) per body ----
        with tc.For_i(0, 8 * T, 8 * UNROLL,
                      hint_engines=(mybir.EngineType.PE,)) as iv0:
            # lookahead-2 chunk DMAs: quarter q produces chunk c+q+2 into
            # pres[(q+2)%4] (c = body's first chunk)
            xts_q = [dma_xchunk((q + 2) % 4, iv0 + 8 * CH * (q + 2))
                     for q in range(4)]
            pps = None
            for u in range(UNROLL):
                q = u // CH
                if u % 4 == 0:
                    pps = ppsum.tile([128, 512], F32, tag="pj", name="pps")
                emit_step(u, iv0 + 8 * u, xts_q[q], pres[(q + 2) % 4], pps)


# ----------------------------------------------------------------------------
# Host side
# ----------------------------------------------------------------------------

_PROG_CACHE = {}


def _get_program(T):
    if T not in _PROG_CACHE:
        _PROG_CACHE[T] = build_program(T)
    return _PROG_CACHE[T]


def _bf16(a):
    import ml_dtypes
    return np.asarray(a, np.float32).astype(ml_dtypes.bfloat16)


def _fp8(a):
    import ml_dtypes
    return np.asarray(a, np.float32).astype(ml_dtypes.float8_e4m3)


SCALE = 16.0


def _chunkT(w, dtype_fn=_bf16, scale=SCALE):
    """[G4, K] weight -> scaled W^T [K//128, 128, G4]."""
    wt = np.ascontiguousarray(w.T.astype(np.float32)) * scale
    return dtype_fn(wt.reshape(wt.shape[0] // 128, 128, w.shape[0]))


def _biasT(b_ih, b_hh, scale=SCALE):
    """-> [128, 16] f32 per-mslot per-partition bias (x scale)."""
    b = (np.asarray(b_ih, np.float32) + np.asarray(b_hh, np.float32)) * scale
    out = np.zeros((128, 16), np.float32)
    for s, m in enumerate(MORDER):
        out[:, s] = b[128 * m:128 * (m + 1)]
    return out


def _prep_inputs(x, w_ih_f0, w_hh_f0, b_ih_f0, b_hh_f0,
                 w_ih_b0, w_hh_b0, b_ih_b0, b_hh_b0,
                 w_ih_f1, w_hh_f1, b_ih_f1, b_hh_f1,
                 w_ih_b1, w_hh_b1, b_ih_b1, b_hh_b1,
                 mask, T):
    f32 = np.float32
    wq = _fp8 if USE_FP8 else _bf16
    whh_f0m = (w_hh_f0 * mask).astype(f32)

    per_dir = {
        0: dict(w0T=_chunkT(w_ih_f0), b0T=_biasT(b_ih_f0, b_hh_f0),
                whh0T=_chunkT(whh_f0m, wq),
                w1oT=_chunkT(w_ih_f1[:, :H]), w1pT=_chunkT(w_ih_f1[:, H:]),
                b1T=_biasT(b_ih_f1, b_hh_f1),
                whh1T=_chunkT(w_hh_f1, wq)),
        1: dict(w0T=_chunkT(w_ih_b0), b0T=_biasT(b_ih_b0, b_hh_b0),
                whh0T=_chunkT(w_hh_b0, wq),
                w1oT=_chunkT(w_ih_b1[:, H:]), w1pT=_chunkT(w_ih_b1[:, :H]),
                b1T=_biasT(b_ih_b1, b_hh_b1),
                whh1T=_chunkT(w_hh_b1, wq)),
    }

    TB = T * BS
    in_maps = []
    for core in range(NCORES):
        pair, q = core // 2, core % 2
        xs = x[pair * BS:(pair + 1) * BS, :T].astype(f32)   # [8, T, E]
        if q == 1:
            xs = xs[:, ::-1]
        xT = np.ascontiguousarray(xs.transpose(2, 1, 0)).reshape(4, 128, TB)
        xTp = np.zeros((4, 128, TB + PAD), np.float32)
        xTp[:, :, :TB] = xT
        m = dict(per_dir[q])
        m["x_T"] = _bf16(xTp)
        m["pslot"] = np.array([[1 - q]], dtype=np.uint32)
        in_maps.append(m)
    return in_maps


def _mish(x):
    return x * np.tanh(np.log1p(np.exp(-np.abs(x))) + np.maximum(x, 0.0))


def _unT(hT):
    """[128, 32] h^T-chunk layout -> [8, 512]: h[b, 128k+p] = hT[p, 8k+b]."""
    a = np.asarray(hT, np.float32).reshape(128, 4, 8)     # p, k, b
    return np.ascontiguousarray(a.transpose(2, 1, 0)).reshape(8, 512)


def _head(h1s, fc_w, fc_b):
    h1s = [_unT(h) for h in h1s]
    h_f = np.concatenate([np.asarray(h1s[2 * p], np.float32) for p in range(4)], axis=0)
    h_b = np.concatenate([np.asarray(h1s[2 * p + 1], np.float32) for p in range(4)], axis=0)
    h = 0.5 * (h_f + h_b)
    z = h @ np.asarray(fc_w, np.float32).T + np.asarray(fc_b, np.float32)
    return _mish(z).astype(np.float32)


def run_device(inputs, trace=False, tmpdir=None):
    from concourse.bass_utils import run_bass_kernel_spmd

    x = inputs["x"]
    T = x.shape[1]
    nc = _get_program(T)
    in_maps = _prep_inputs(
        x, inputs["w_ih_f0"], inputs["w_hh_f0"], inputs["b_ih_f0"], inputs["b_hh_f0"],
        inputs["w_ih_b0"], inputs["w_hh_b0"], inputs["b_ih_b0"], inputs["b_hh_b0"],
        inputs["w_ih_f1"], inputs["w_hh_f1"], inputs["b_ih_f1"], inputs["b_hh_f1"],
        inputs["w_ih_b1"], inputs["w_hh_b1"], inputs["b_ih_b1"], inputs["b_hh_b1"],
        inputs["mask"], T)

    res = run_bass_kernel_spmd(nc, in_maps, list(range(NCORES)),
                               trace=trace, tmpdir=tmpdir)
    h1s = [res.results[c]["h1"] for c in range(NCORES)]
    return h1s, res


def kernel(**inputs):
    h1s, _ = run_device(inputs)
    return _head(h1s, inputs["fc_w"], inputs["fc_b"])



# revision 3
# speedup vs baseline: 1.0005x; 1.0005x over previous
"""AWD-LSTM forward on 8 Trainium2 NeuronCores — v2 "warm" design.

Sharding (unchanged from v1): 8 cores = 4 pairs; even core of a pair runs
the forward direction, odd the backward (fed time-reversed inputs so both
are forward scans).  Batch (32) is sharded 8 rows per pair.  Layer-1 input
needs both directions' layer-0 sequences, exchanged with a pairwise
AllGather.  The tiny fc+mish head runs on host.

v2 changes (vs the 16.3 ms v1):
  - The input projections (pre = x @ W_ih^T + bias) are FUSED into the
    recurrence loop: each step's tail (the ACT/DVE gate chain, ~1.5 us of
    PE idle in v1) is filled with a few projection matmuls that compute
    pre for steps ~16-32 ahead, into SBUF ping-pong chunk buffers.  This
    keeps the PE busy so the HAM clock gate stays at K=8/8 (2.4 GHz) —
    v1 ran throttled at 1.2 GHz for 94% of the kernel — and absorbs the
    ~0.7 ms standalone projection phases.
  - The 16 per-step pre-fold transpose-matmuls are gone: pre enters the
    gate PSUM banks via a DVE value-overwrite while the accumulation
    group stays open forever (one dummy start=True matmul at init; all
    gate matmuls start=False).  has_written bits stay set, so matmuls
    accumulate on top of the DVE-written pre values.
  - W_hh is fp8-e4m3 (stationary side only; h stays bf16): FWL loads
    fp8 weights 2x faster than bf16.  Everything is pre-scaled by 16 on
    host (clears e4m3's subnormal floor) and descaled for free via the
    ACT affine (scale=1/16).  Measured fp8 impact on final error vs
    fp32 reference: 6.7e-3 (budget 2e-2).
  - Gate banks laid out [i|g] and [f|o] so the exposed tail chain is
    sigmoid(f,o) -> f*c -> +i*tanh(g) -> tanh(c) -> o*tanh(c), with the
    i/g activations hidden under the f/o matmuls.
"""

import sys

sys.path.insert(0, "/opt/trn_rl_repo")

import numpy as np

import concourse.bass as bass
import concourse.bacc as bacc
import concourse.mybir as mybir
import concourse.tile as tile
from concourse.bass import ds

B, S, E = 32, 1024, 512
H = E
G4 = 4 * H            # 2048
BS = 8                # batch rows per pair
NCORES = 8
CH = 16               # steps per pre chunk
UNROLL = 64           # steps per loop body (4 chunks)
PAD = 2 * CH * BS     # x/seq column padding (2 chunks of lookahead slop)
F32 = mybir.dt.float32
BF16 = mybir.dt.bfloat16
FP8 = mybir.dt.float8e4
USE_FP8 = True

AF = mybir.ActivationFunctionType
MULT = mybir.AluOpType.mult
ADD = mybir.AluOpType.add

# mslot s (production/bias/pre-buffer order) -> real m-chunk (gate dim / 128)
# slots 0-7 -> bank_ig [i0..3, g0..3]; slots 8-15 -> bank_fo [f0..3, o0..3]
MORDER = [0, 1, 2, 3, 8, 9, 10, 11, 4, 5, 6, 7, 12, 13, 14, 15]


def build_program(T=S, fp8=USE_FP8):
    TB = T * BS
    TBP = TB + PAD
    WDT = FP8 if fp8 else BF16
    nc = bacc.Bacc(None, target_bir_lowering=False)

    # ---- I/O ----
    x_T = nc.dram_tensor("x_T", [4, 128, TBP], BF16, kind="ExternalInput")
    w0T = nc.dram_tensor("w0T", [4, 128, G4], BF16, kind="ExternalInput")
    b0T = nc.dram_tensor("b0T", [128, 16], F32, kind="ExternalInput")
    whh0T = nc.dram_tensor("whh0T", [4, 128, G4], WDT, kind="ExternalInput")
    w1oT = nc.dram_tensor("w1oT", [4, 128, G4], BF16, kind="ExternalInput")
    w1pT = nc.dram_tensor("w1pT", [4, 128, G4], BF16, kind="ExternalInput")
    b1T = nc.dram_tensor("b1T", [128, 16], F32, kind="ExternalInput")
    whh1T = nc.dram_tensor("whh1T", [4, 128, G4], WDT, kind="ExternalInput")
    pslot = nc.dram_tensor("pslot", [1, 1], mybir.dt.uint32, kind="ExternalInput")
    h1 = nc.dram_tensor("h1", [128, 32], BF16, kind="ExternalOutput")
    dbg_pre = nc.dram_tensor("dbg_pre", [128, 16, CH * BS], BF16, kind="ExternalOutput")
    dbg_loc = nc.dram_tensor("dbg_loc", [128, 4, 128], BF16, kind="ExternalOutput")

    with tile.TileContext(nc) as tc:
        with tc.tile_pool(name="dram", bufs=1, space="DRAM") as dram:
            locT = dram.tile([128, 4, TBP], BF16)
            revT = dram.tile([128, 4, TBP], BF16)
            ag = dram.tile([2, 128, 4, TBP], BF16)
            pbuf = dram.tile([128, 4, TBP], BF16)

            with (
                tc.tile_pool(name="outer", bufs=1) as op,
                tc.tile_pool(name="gpsum", bufs=1, space="PSUM") as gpsum,
                tc.tile_pool(name="ppsum", bufs=2, space="PSUM") as ppsum,
                tc.tile_pool(name="gtmp", bufs=2) as gp,
                tc.tile_pool(name="stage", bufs=4) as stp,
                tc.tile_pool(name="prebuf", bufs=1) as prep,
                tc.tile_pool(name="xin", bufs=2) as xp,
            ):
                bank_ig = gpsum.tile([128, 64], F32)
                bank_fo = gpsum.tile([128, 64], F32)
                cT = op.tile([128, 32], F32)
                hT = op.tile([128, 32], BF16)
                pres = [prep.tile([128, 16, CH * BS], BF16, name=f"pre{i}")
                        for i in range(4)]
                zlhs = op.tile([1, 128], BF16)
                zrhs = op.tile([1, 64], BF16)
                zslop = op.tile([128, 4, PAD], BF16)
                nc.gpsimd.memset(zlhs[:], 0.0)
                nc.gpsimd.memset(zrhs[:], 0.0)
                nc.gpsimd.memset(zslop[:], 0.0)
                nc.gpsimd.memset(cT[:], 0.0)
                nc.gpsimd.memset(hT[:], 0.0)

                # open the gate PSUM accumulation groups forever: write 0s with
                # start=True (sets has_written over the full [128, 64] region),
                # never issue stop.  All gate matmuls accumulate (start=False)
                # on top of DVE-prewritten pre values.
                nc.tensor.matmul(bank_ig[:], zlhs[:], zrhs[:],
                                 start=True, stop=False, skip_group_check=True)
                nc.tensor.matmul(bank_fo[:], zlhs[:], zrhs[:],
                                 start=True, stop=False, skip_group_check=True)

                # zero the lookahead slop so layer-1's projection reads are finite
                nc.sync.dma_start(locT[:, :, TB:TBP], zslop[:])
                nc.sync.dma_start(revT[:, :, TB:TBP], zslop[:])

                pools = dict(op=op, gp=gp, stp=stp, xp=xp, ppsum=ppsum,
                             bank_ig=bank_ig, bank_fo=bank_fo, cT=cT, hT=hT,
                             pres=pres)

                recur_layer(tc, pools, T, WDT,
                            xsrc=[(x_T, k) for k in range(4)],
                            wihT=[(w0T, k) for k in range(4)],
                            whhT=whh0T, biasT=b0T,
                            locT=locT, revT=revT, dbg_pre=dbg_pre)
                nc.sync.dma_start(dbg_loc[:], locT[:, :, 0:128])

                nc.gpsimd.collective_compute(
                    "AllGather",
                    mybir.AluOpType.bypass,
                    ins=[revT.opt()],
                    outs=[ag.opt()],
                    replica_groups=[[0, 1], [2, 3], [4, 5], [6, 7]],
                )

                # copy partner's gathered (pre-reversed) sequence to pbuf
                with tc.tile_pool(name="pslot", bufs=1) as pp:
                    pslot_sb = pp.tile([1, 1], mybir.dt.uint32)
                    nc.sync.dma_start(pslot_sb[:], pslot[:])
                    tmp_reg = nc.sync.alloc_register("pslot_reg")
                    nc.sync.reg_load(tmp_reg, pslot_sb[0:1, 0:1])
                    slot_reg = nc.sync.snap(tmp_reg, donate=True, min_val=0, max_val=1)
                    nc.sync.dma_start(
                        pbuf[:],
                        ag[ds(slot_reg, 1)].rearrange("s p k c -> (s p) k c"),
                    )

                nc.gpsimd.memset(cT[:], 0.0)
                nc.gpsimd.memset(hT[:], 0.0)

                recur_layer(tc, pools, T, WDT,
                            xsrc=[(locT, k) for k in range(4)]
                                 + [(pbuf, k) for k in range(4)],
                            wihT=[(w1oT, k) for k in range(4)]
                                 + [(w1pT, k) for k in range(4)],
                            whhT=whh1T, biasT=b1T)

                nc.sync.dma_start(h1[:], hT[:])

    nc.compile()
    return nc


def _xslice(src, k, col, n=128):
    """[128, n] moving slice at column `col` from an x-like source."""
    t, kk = src
    if len(t.shape) == 3 and t.shape[0] == 4:     # x_T [4, 128, TBP]
        return t[kk, :, col] if isinstance(col, slice) else t[kk, :, ds(col, n)]
    # locT/pbuf [128, 4, TBP]
    return t[:, kk, col] if isinstance(col, slice) else t[:, kk, ds(col, n)]


def recur_layer(tc, P, T, WDT, xsrc, wihT, whhT, biasT, locT=None, revT=None,
                dbg_pre=None):
    """One LSTM layer: fused projection + recurrence.

    xsrc: list of (dram_tensor, k) moving-operand sources, one per 128-wide
    contraction chunk (4 for layer 0, 4 own + 4 partner for layer 1).
    """
    nc = tc.nc
    nk = len(xsrc)
    store = locT is not None
    bank_ig, bank_fo = P["bank_ig"], P["bank_fo"]
    cT, hT, pres = P["cT"], P["hT"], P["pres"]
    gp, stp, xp, ppsum, op = P["gp"], P["stp"], P["xp"], P["ppsum"], P["op"]
    REV = 8 * T - 8

    with tc.tile_pool(name="wpool", bufs=1) as wp:
        whh_sb = []
        for k in range(4):
            w = wp.tile([128, G4], WDT, name=f"whh{k}")
            nc.sync.dma_start(w[:], whhT[k])
            whh_sb.append(w)
        wih_sb = []
        for j, (t, kk) in enumerate(wihT):
            w = wp.tile([128, G4], BF16, name=f"wih{j}")
            nc.sync.dma_start(w[:], t[kk])
            wih_sb.append(w)
        bias_sb = wp.tile([128, 16], F32)
        nc.sync.dma_start(bias_sb[:], biasT[:])

        def dma_xchunk(parity, col):
            """Fetch the [128,128] moving tiles for one chunk's projection."""
            xts = []
            for j in range(nk):
                xt = xp.tile([128, 128], BF16, name=f"x{j}", tag=f"x{j}p{parity}")
                nc.sync.dma_start(xt[:], _xslice(xsrc[j], j, col))
                xts.append(xt)
            return xts

        def emit_proj(s, xts, dst, pps):
            """Projection matmuls for mslot s into pps col 128*(s%4)."""
            m = MORDER[s]
            col = 128 * (s % 4)
            for j in range(nk):
                nc.tensor.matmul(pps[:, col:col + 128],
                                 wih_sb[j][:, 128 * m:128 * (m + 1)], xts[j][:],
                                 start=(j == 0), stop=(j == nk - 1))

        def emit_proj_copies(g, dst, pps):
            """Evacuate mslots 4g..4g+3 (+bias) from pps into dst."""
            for jj in range(4):
                s = 4 * g + jj
                nc.vector.tensor_scalar_add(dst[:, s, :],
                                            pps[:, 128 * jj:128 * jj + 128],
                                            bias_sb[:, s:s + 1])

        def prewrite(nxt_s):
            """DVE-overwrite both gate banks with pre for step nxt_s."""
            buf = pres[(nxt_s // CH) % 4]
            c0 = 8 * (nxt_s % CH)
            nc.vector.tensor_copy(
                bank_ig[:].rearrange("p (s b) -> p s b", s=8),
                buf[:, 0:8, c0:c0 + 8])
            nc.vector.tensor_copy(
                bank_fo[:].rearrange("p (s b) -> p s b", s=8),
                buf[:, 8:16, c0:c0 + 8])

        def emit_step(u, iv8, xts, dst, pps):
            """One LSTM step; consumes prewritten banks, prewrites step u+1.

            u: step index within the body (0..UNROLL-1); also emits the
            projection for mslot (u%CH) of the lookahead chunk into dst.
            """
            # Order [i, g, f, o] with sigmoid(f) SPLIT from sigmoid(o):
            # the per-MM sem-inc stream drains ~41ns/position, so each gate's
            # consumer starts at its block's last stream position.  With f at
            # positions 33-48 and o at 49-64, sig_f -> btmp -> cT -> tanh(c)
            # all finish before the o-block inc; only sig_o -> h stays exposed.
            for j in range(4):
                for k in range(4):
                    nc.tensor.matmul(bank_ig[:, 8 * j:8 * j + 8],
                                     whh_sb[k][:, 128 * j:128 * (j + 1)],
                                     hT[:, 8 * k:8 * k + 8],
                                     start=False, stop=False, skip_group_check=True)
            si = gp.tile([128, 32], F32, tag="si")
            nc.scalar.activation(si[:], bank_ig[:, 0:32], AF.Sigmoid, scale=0.0625)
            for j in range(4):
                m = 8 + j
                for k in range(4):
                    nc.tensor.matmul(bank_ig[:, 32 + 8 * j:40 + 8 * j],
                                     whh_sb[k][:, 128 * m:128 * (m + 1)],
                                     hT[:, 8 * k:8 * k + 8],
                                     start=False, stop=False, skip_group_check=True)
            tg = gp.tile([128, 32], F32, tag="tg")
            nc.scalar.activation(tg[:], bank_ig[:, 32:64], AF.Tanh, scale=0.0625)
            for j in range(4):
                m = 4 + j
                for k in range(4):
                    nc.tensor.matmul(bank_fo[:, 8 * j:8 * j + 8],
                                     whh_sb[k][:, 128 * m:128 * (m + 1)],
                                     hT[:, 8 * k:8 * k + 8],
                                     start=False, stop=False, skip_group_check=True)
            sf = gp.tile([128, 32], F32, tag="sf")
            nc.scalar.activation(sf[:], bank_fo[:, 0:32], AF.Sigmoid, scale=0.0625)
            for j in range(4):
                m = 12 + j
                for k in range(4):
                    nc.tensor.matmul(bank_fo[:, 32 + 8 * j:40 + 8 * j],
                                     whh_sb[k][:, 128 * m:128 * (m + 1)],
                                     hT[:, 8 * k:8 * k + 8],
                                     start=False, stop=False, skip_group_check=True)
            so = gp.tile([128, 32], F32, tag="so")
            nc.scalar.activation(so[:], bank_fo[:, 32:64], AF.Sigmoid, scale=0.0625)

            buf = pres[((u + 1) // CH) % 4]
            c0 = 8 * ((u + 1) % CH)
            a = gp.tile([128, 32], F32, tag="a")
            nc.vector.tensor_tensor(a[:], si[:], tg[:], MULT)
            nc.vector.tensor_copy(
                bank_ig[:].rearrange("p (s b) -> p s b", s=8),
                buf[:, 0:8, c0:c0 + 8])
            btmp = gp.tile([128, 32], F32, tag="btmp")
            nc.vector.tensor_tensor(btmp[:], sf[:], cT[:], MULT)
            nc.vector.tensor_tensor(cT[:], a[:], btmp[:], ADD)
            emit_proj(u % CH, xts, dst, pps)
            if u % 4 == 3:
                emit_proj_copies((u % CH) // 4, dst, pps)
            tct = gp.tile([128, 32], F32, tag="tct")
            nc.scalar.activation(tct[:], cT[:], AF.Tanh)
            nc.vector.tensor_tensor(hT[:], so[:], tct[:], MULT)
            nc.vector.tensor_copy(
                bank_fo[:].rearrange("p (s b) -> p s b", s=8),
                buf[:, 8:16, c0:c0 + 8])

            if store:
                st = stp.tile([128, 32], BF16, tag="st")
                nc.gpsimd.tensor_copy(st[:], hT[:])
                st3 = st[:].rearrange("p (k b) -> p k b", k=4)
                nc.sync.dma_start(locT[:, :, ds(iv8, 8)], st3)
                nc.gpsimd.dma_start(revT[:, :, ds(REV - iv8, 8)], st3)

        # ---- prologue: produce chunks 0-1 into pres[0:2], prewrite step 0 ----
        for c in range(2):
            xtsp = dma_xchunk(c, slice(128 * c, 128 * c + 128))
            for g in range(4):
                pps = ppsum.tile([128, 512], F32, tag="pj", name="pps")
                for jj in range(4):
                    emit_proj(4 * g + jj, xtsp, pres[c], pps)
                emit_proj_copies(g, pres[c], pps)
        prewrite(0)
        if dbg_pre is not None:
            nc.sync.dma_start(dbg_pre[:], pres[0][:])

        # ---- main loop: 32 steps (2 chunks) per body ----
        with tc.For_i(0, 8 * T, 8 * UNROLL,
                      hint_engines=(mybir.EngineType.PE,)) as iv0:
            # lookahead-2 chunk DMAs: quarter q produces chunk c+q+2 into
            # pres[(q+2)%4] (c = body's first chunk)
            xts_q = [dma_xchunk((q + 2) % 4, iv0 + 8 * CH * (q + 2))
                     for q in range(4)]
            pps = None
            for u in range(UNROLL):
                q = u // CH
                if u % 4 == 0:
                    pps = ppsum.tile([128, 512], F32, tag="pj", name="pps")
                emit_step(u, iv0 + 8 * u, xts_q[q], pres[(q + 2) % 4], pps)


# ----------------------------------------------------------------------------
# Host side
# ----------------------------------------------------------------------------

_PROG_CACHE = {}


def _get_program(T):
    if T not in _PROG_CACHE:
        _PROG_CACHE[T] = build_program(T)
    return _PROG_CACHE[T]


def _bf16(a):
    import ml_dtypes
    return np.asarray(a, np.float32).astype(ml_dtypes.bfloat16)


def _fp8(a):
    import ml_dtypes
    return np.asarray(a, np.float32).astype(ml_dtypes.float8_e4m3)


SCALE = 16.0


def _chunkT(w, dtype_fn=_bf16, scale=SCALE):
    """[G4, K] weight -> scaled W^T [K//128, 128, G4]."""
    wt = np.ascontiguousarray(w.T.astype(np.float32)) * scale
    return dtype_fn(wt.reshape(wt.shape[0] // 128, 128, w.shape[0]))


def _biasT(b_ih, b_hh, scale=SCALE):
    """-> [128, 16] f32 per-mslot per-partition bias (x scale)."""
    b = (np.asarray(b_ih, np.float32) + np.asarray(b_hh, np.float32)) * scale
    out = np.zeros((128, 16), np.float32)
    for s, m in enumerate(MORDER):
        out[:, s] = b[128 * m:128 * (m + 1)]
    return out


def _prep_inputs(x, w_ih_f0, w_hh_f0, b_ih_f0, b_hh_f0,
                 w_ih_b0, w_hh_b0, b_ih_b0, b_hh_b0,
                 w_ih_f1, w_hh_f1, b_ih_f1, b_hh_f1,
                 w_ih_b1, w_hh_b1, b_ih_b1, b_hh_b1,
                 mask, T):
    f32 = np.float32
    wq = _fp8 if USE_FP8 else _bf16
    whh_f0m = (w_hh_f0 * mask).astype(f32)

    per_dir = {
        0: dict(w0T=_chunkT(w_ih_f0), b0T=_biasT(b_ih_f0, b_hh_f0),
                whh0T=_chunkT(whh_f0m, wq),
                w1oT=_chunkT(w_ih_f1[:, :H]), w1pT=_chunkT(w_ih_f1[:, H:]),
                b1T=_biasT(b_ih_f1, b_hh_f1),
                whh1T=_chunkT(w_hh_f1, wq)),
        1: dict(w0T=_chunkT(w_ih_b0), b0T=_biasT(b_ih_b0, b_hh_b0),
                whh0T=_chunkT(w_hh_b0, wq),
                w1oT=_chunkT(w_ih_b1[:, H:]), w1pT=_chunkT(w_ih_b1[:, :H]),
                b1T=_biasT(b_ih_b1, b_hh_b1),
                whh1T=_chunkT(w_hh_b1, wq)),
    }

    TB = T * BS
    in_maps = []
    for core in range(NCORES):
        pair, q = core // 2, core % 2
        xs = x[pair * BS:(pair + 1) * BS, :T].astype(f32)   # [8, T, E]
        if q == 1:
            xs = xs[:, ::-1]
        xT = np.ascontiguousarray(xs.transpose(2, 1, 0)).reshape(4, 128, TB)
        xTp = np.zeros((4, 128, TB + PAD), np.float32)
        xTp[:, :, :TB] = xT
        m = dict(per_dir[q])
        m["x_T"] = _bf16(xTp)
        m["pslot"] = np.array([[1 - q]], dtype=np.uint32)
        in_maps.append(m)
    return in_maps


def _mish(x):
    return x * np.tanh(np.log1p(np.exp(-np.abs(x))) + np.maximum(x, 0.0))


def _unT(hT):
    """[128, 32] h^T-chunk layout -> [8, 512]: h[b, 128k+p] = hT[p, 8k+b]."""
    a = np.asarray(hT, np.float32).reshape(128, 4, 8)     # p, k, b
    return np.ascontiguousarray(a.transpose(2, 1, 0)).reshape(8, 512)


def _head(h1s, fc_w, fc_b):
    h1s = [_unT(h) for h in h1s]
    h_f = np.concatenate([np.asarray(h1s[2 * p], np.float32) for p in range(4)], axis=0)
    h_b = np.concatenate([np.asarray(h1s[2 * p + 1], np.float32) for p in range(4)], axis=0)
    h = 0.5 * (h_f + h_b)
    z = h @ np.asarray(fc_w, np.float32).T + np.asarray(fc_b, np.float32)
    return _mish(z).astype(np.float32)


def run_device(inputs, trace=False, tmpdir=None):
    from concourse.bass_utils import run_bass_kernel_spmd

    x = inputs["x"]
    T = x.shape[1]
    nc = _get_program(T)
    in_maps = _prep_inputs(
        x, inputs["w_ih_f0"], inputs["w_hh_f0"], inputs["b_ih_f0"], inputs["b_hh_f0"],
        inputs["w_ih_b0"], inputs["w_hh_b0"], inputs["b_ih_b0"], inputs["b_hh_b0"],
        inputs["w_ih_f1"], inputs["w_hh_f1"], inputs["b_ih_f1"], inputs["b_hh_f1"],
        inputs["w_ih_b1"], inputs["w_hh_b1"], inputs["b_ih_b1"], inputs["b_hh_b1"],
        inputs["mask"], T)

    res = run_bass_kernel_spmd(nc, in_maps, list(range(NCORES)),
                               trace=trace, tmpdir=tmpdir)
    h1s = [res.results[c]["h1"] for c in range(NCORES)]
    return h1s, res


def kernel(**inputs):
    h1s, _ = run_device(inputs)
    return _head(h1s, inputs["fc_w"], inputs["fc_b"])

